# revision 15
# baseline (speedup 1.0000x reference)
"""BinaryTreeLSTM on 8 TRN2 NeuronCores.

Strategy: tensor-parallel over the 8H gate dimension (sharding hint).
Key algebraic facts exploited:
  - The reference keeps only the first H dims of h_new/c_new per level, so
    only gate rows {q*2H + [0:H]} of the 8H weight rows ever matter
    ("kept gates": 4H instead of 8H -> 2x less matmul work).
  - c_cat[:, :H] is the LEFT child's c only, elementwise per hidden dim ->
    c never needs to be exchanged between cores; only h is all-gathered.
  - At the leaf level h = c = 0 -> the W_hh matmul and the f-gate*c term
    are skipped entirely.
Each core m owns hidden dims [128m, 128m+128) of each of the i,f,g,o gates
(a 512-wide gate slice). Per level it computes gates.T (feature-major:
gate dims on PSUM partitions, nodes on the free axis), applies the LSTM
cell elementwise, and all-gathers its h.T slice (128, n) into the full
h.T (1024, n) for the next level.

v2 vs baseline (433749 ns):
  - h stored/gathered as float8e3 (e3m4) scaled x8 -> AllGather bytes
    halved (the big leaf gathers were ~86us of serial cc time).
  - W_hh stored as e3m4 scaled x64 -> hh matmuls are fp8xfp8, half the
    LDWEIGHTS time, half the SBUF.  Gates come out x512; the scalar-engine
    activation computes func(in*1/512 + bias) for free.
  - x-path stays bf16 (fp8 there alone costs 2.7e-2 rel err - fails).
    Numpy end-to-end sim of this exact mix: rel err 7.9e-3 (budget 2e-2).
  - A dummy 1-column AllGather issued first absorbs the one-time ~38us
    collective BARRIER behind the weight DMAs + leaf compute.
"""

import sys

for p in ("/opt/trn_rl_repo",):
    if p not in sys.path:
        sys.path.insert(0, p)

import numpy as np

import concourse.bass as bass
import concourse.bacc as bacc
import concourse.mybir as mybir
import concourse.tile as tile
from concourse import bass_utils

H = 1024
I = 1024
DEPTH = 12
NCORES = 8
P = 128            # partitions / per-core hidden slice
GS = 4 * P         # per-core gate slice (i,f,g,o each P wide) = 512
NCHUNK = 512       # node-column chunk (PSUM bank = 512 fp32)
F32 = mybir.dt.float32
BF16 = mybir.dt.bfloat16
F8E3 = mybir.dt.float8e3
AF = mybir.ActivationFunctionType

W_HH_SCALE = 64.0   # W_hh quantization scale (e3m4)
H_SCALE = 8.0       # h quantization scale (e3m4)
G_SCALE = W_HH_SCALE * H_SCALE   # gates arrive x512 -> activation scale 1/512

_CACHE = {}


def _build():
    nc = bacc.Bacc(
        "TRN2",
        target_bir_lowering=False,
        debug=False,
        enable_asserts=False,
        num_devices=NCORES,
    )

    NTOT = 2 ** DEPTH - 1  # 4095
    embT_d = nc.dram_tensor("embT", (I, NTOT), BF16, kind="ExternalInput")
    embS_d = nc.dram_tensor("embS", (I, 128), BF16, kind="ExternalInput")
    wihT_d = nc.dram_tensor("wihT", (I, GS), BF16, kind="ExternalInput")
    whhT_d = nc.dram_tensor("whhT", (2 * H, GS), F8E3, kind="ExternalInput")
    bias_d = nc.dram_tensor("bias", (P, 4), F32, kind="ExternalInput")
    brow_d = nc.dram_tensor("brow", (1, GS), BF16, kind="ExternalInput")
    ones_d = nc.dram_tensor("ones", (1, P), BF16, kind="ExternalInput")
    eye_d = nc.dram_tensor("eye", (P, P), F32, kind="ExternalInput")
    out_d = nc.dram_tensor("out", (2 * P, 1), F32, kind="ExternalOutput")

    KX = I // P        # 8 contraction chunks for the x part
    KH = 2 * H // P    # 16 contraction chunks for the hh part
    rg = [list(range(NCORES))]

    with tile.TileContext(nc) as tc:
        with (
            tc.tile_pool(name="wpool", bufs=1) as wpool,
            tc.tile_pool(name="xpool", bufs=2) as xpool,
            tc.tile_pool(name="spool", bufs=2) as spool,
            tc.tile_pool(name="state", bufs=2) as state,
            tc.tile_pool(name="ewpool", bufs=2) as ewpool,
            tc.tile_pool(name="psum", bufs=8, space=bass.MemorySpace.PSUM) as psum,
            tc.tile_pool(name="dram", bufs=2, space=bass.MemorySpace.DRAM) as dram,
        ):
            # ---- dummy collective: absorb the one-time cc BARRIER ----
            dummy_in = dram.tile([P, 2], F32, name="dummy_in")
            dummy_out = dram.tile([NCORES * P, 2], F32, name="dummy_out",
                                  addr_space="Shared")
            nc.sync.dma_start(dummy_in[:], bias_d[:, 0:2])
            nc.gpsimd.collective_compute(
                "AllGather",
                mybir.AluOpType.bypass,
                replica_groups=rg,
                ins=[dummy_in.opt()],
                outs=[dummy_out.opt()],
            )

            # resident weights, feature-major: [:, c, q*128:(q+1)*128] is the
            # stationary (K=128, M=128) tile for contraction chunk c, gate q
            wih = wpool.tile([P, KX, GS], BF16)
            whh = wpool.tile([P, KH, GS], F8E3)
            bias = wpool.tile([P, 4], F32)
            brow = wpool.tile([1, GS], BF16)
            ones = wpool.tile([1, P], BF16)
            eye = wpool.tile([P, P], F32)
            for a in range(KX):
                nc.sync.dma_start(wih[:, a, :], wihT_d[a * P:(a + 1) * P, :])
            for c in range(KH):
                nc.scalar.dma_start(whh[:, c, :], whhT_d[c * P:(c + 1) * P, :])
            nc.sync.dma_start(bias[:], bias_d[:])
            nc.gpsimd.dma_start(brow[:], brow_d[:])
            nc.gpsimd.dma_start(ones[:], ones_d[:])
            nc.gpsimd.dma_start(eye[:], eye_d[:])

            # ---- phase structure ------------------------------------
            # 1. leaf level (k=11): x-only gates, elementwise, chunked
            #    AllGathers start flowing immediately.
            # 2. XW precompute: x@W_ih.T for ALL interior nodes (heap rows
            #    0..2046) into SBUF - dense PE work that runs while the
            #    leaf AllGathers drain, and removes the x part from the
            #    recurrent critical path entirely.  Stored x512 to match
            #    the scaled hh PSUM.
            # 3. levels 10..0: hh-only PSUM groups + xw combine + cell.
            xw = wpool.tile([P, 4, 2 * NCHUNK * 2], F32)  # (128, 4, 2048)
            # emb.T x512 for the tiny top-tree levels (heap rows 0..126)
            exS = wpool.tile([P, KX, 128], BF16)

            lvl = {}

            def get_level(k):
                if k not in lvl:
                    n = 2 ** k
                    h_new = state.tile(
                        [P, max(n, 2)], F8E3, tag="hst", bufs=2, name=f"h{k}"
                    )
                    c_new = state.tile(
                        [P, max(n, 2)], F32, tag="cst", bufs=3, name=f"c{k}"
                    )
                    lvl[k] = {"h": h_new, "c": c_new, "hgat": []}
                return lvl[k]

            def emit_tail(k, j0, w, wp, ps, gscale, gather=True):
                """LSTM cell on finished gate tiles + chunked AllGather.

                ps tiles hold gates * (1/gscale is applied here); bias is in
                true units (activation computes func(in*scale + bias)).
                """
                L = lvl[k]
                h_new, c_new = L["h"], L["c"]
                sig_i = ewpool.tile([P, wp], F32, tag="si")
                tan_g = ewpool.tile([P, wp], F32, tag="tg")
                sig_o = ewpool.tile([P, wp], F32, tag="so")
                nc.scalar.activation(sig_i[:], ps[0][:], AF.Sigmoid,
                                     bias=bias[:, 0:1], scale=gscale)
                nc.scalar.activation(tan_g[:], ps[2][:], AF.Tanh,
                                     bias=bias[:, 2:3], scale=gscale)
                if k < DEPTH - 1:
                    sig_f = ewpool.tile([P, wp], F32, tag="sf")
                    nc.scalar.activation(sig_f[:], ps[1][:], AF.Sigmoid,
                                         bias=bias[:, 1:2], scale=gscale)
                nc.scalar.activation(sig_o[:], ps[3][:], AF.Sigmoid,
                                     bias=bias[:, 3:4], scale=gscale)

                t2 = ewpool.tile([P, wp], F32, tag="t2")
                nc.vector.tensor_mul(t2[:], sig_i[:], tan_g[:])
                if k < DEPTH - 1:
                    c_prev = lvl[k + 1]["c"]
                    if wp == w:
                        c_left = c_prev[:, 2 * j0: 2 * j0 + 2 * w: 2]
                    else:
                        c_left = c_prev[:, 0:2]
                    t1 = ewpool.tile([P, wp], F32, tag="t1")
                    nc.vector.tensor_mul(t1[:], sig_f[:], c_left)
                    nc.vector.tensor_add(c_new[:, j0:j0 + wp], t1[:], t2[:])
                else:
                    nc.vector.tensor_copy(c_new[:, j0:j0 + wp], t2[:])

                tan_c = ewpool.tile([P, wp], F32, tag="tc")
                nc.scalar.activation(tan_c[:], c_new[:, j0:j0 + wp], AF.Tanh)
                if k > 0:
                    t_h = ewpool.tile([P, wp], F32, tag="th")
                    nc.vector.tensor_mul(t_h[:], sig_o[:], tan_c[:])
                    # quantize h*8 -> e3m4 for the gather / hh matmul
                    nc.scalar.activation(h_new[:, j0:j0 + wp], t_h[:],
                                         AF.Copy, scale=H_SCALE)
                    for p0 in (range(j0, j0 + w, 512) if gather else ()):
                        pw = min(512, j0 + w - p0)
                        ag_in = dram.tile([P, pw], F8E3, tag="agin", bufs=6,
                                          name=f"agin{k}_{p0}")
                        ag_out = dram.tile([NCORES * P, pw], F8E3, tag="agout",
                                           bufs=10, name=f"agout{k}_{p0}",
                                           addr_space="Shared")
                        nc.sync.dma_start(ag_in[:], h_new[:, p0:p0 + pw])
                        nc.gpsimd.collective_compute(
                            "AllGather",
                            mybir.AluOpType.bypass,
                            replica_groups=rg,
                            ins=[ag_in.opt()],
                            outs=[ag_out.opt()],
                        )
                        L["hgat"].append((ag_out, pw))
                else:
                    h_root = ewpool.tile([P, 2], F32, tag="hroot")
                    nc.vector.tensor_mul(h_root[:], sig_o[:], tan_c[:])
                    nc.sync.dma_start(out_d[0:P, :], h_root[:, 0:1])
                    nc.sync.dma_start(out_d[P:2 * P, :], c_new[:, 0:1])

            # ---- phase 1: leaf level --------------------------------
            K = DEPTH - 1
            nl = 2 ** K
            get_level(K)
            for j in range(nl // NCHUNK):
                j0 = j * NCHUNK
                ex = xpool.tile([P, KX, NCHUNK], BF16, tag="ex", name=f"exL{j}")
                for a in range(KX):
                    nc.sync.dma_start(
                        ex[:, a, :],
                        embT_d[a * P:(a + 1) * P,
                               nl - 1 + j0: nl - 1 + j0 + NCHUNK],
                    )
                ps = [None] * 4
                for q in (0, 2, 3):
                    ps[q] = psum.tile([P, NCHUNK], F32, tag="ps", bufs=8,
                                      name=f"psL{j}_{q}")
                for q in (0, 2, 3):
                    for a in range(KX):
                        nc.tensor.matmul(
                            ps[q][:], wih[:, a, q * P:(q + 1) * P], ex[:, a, :],
                            start=(a == 0), stop=(a == KX - 1),
                        )
                emit_tail(K, j0, NCHUNK, NCHUNK, ps, 1.0, gather=False)
                if j % 2 == 1:
                    g0 = (j - 1) * NCHUNK
                    gw = 2 * NCHUNK
                    ag_in = dram.tile([P, gw], F8E3, tag="agin", bufs=6,
                                      name=f"aginL_{g0}")
                    ag_out = dram.tile([NCORES * P, gw], F8E3, tag="agout",
                                       bufs=10, name=f"agoutL_{g0}",
                                       addr_space="Shared")
                    nc.sync.dma_start(ag_in[:], lvl[K]["h"][:, g0:g0 + gw])
                    nc.gpsimd.collective_compute(
                        "AllGather",
                        mybir.AluOpType.bypass,
                        replica_groups=rg,
                        ins=[ag_in.opt()],
                        outs=[ag_out.opt()],
                    )
                    lvl[K]["hgat"].append((ag_out, gw))

            for a in range(KX):
                nc.gpsimd.dma_start(exS[:, a, :], embS_d[a * P:(a + 1) * P, :])

            # ---- phase 2: XW precompute for heap rows 0..2047 -------
            for j in range(4):
                j0 = j * NCHUNK
                ex = xpool.tile([P, KX, NCHUNK], BF16, tag="ex", name=f"exP{j}")
                nc.sync.dma_start(
                    ex[:],
                    embT_d[:, j0: j0 + NCHUNK].rearrange("(a p) w -> p a w", p=P),
                )
                for q in range(4):
                    pt = psum.tile([P, NCHUNK], F32, tag="ps", bufs=8,
                                   name=f"psP{j}_{q}")
                    for a in range(KX):
                        nc.tensor.matmul(
                            pt[:], wih[:, a, q * P:(q + 1) * P], ex[:, a, :],
                            start=(a == 0), stop=(a == KX - 1),
                        )
                    # store x512 so it adds directly onto the scaled hh PSUM
                    nc.scalar.activation(xw[:, q, j0:j0 + NCHUNK], pt[:],
                                         AF.Copy, scale=G_SCALE)

            # ---- phase 3: recurrent sweep, hh only ------------------
            for k in range(DEPTH - 2, -1, -1):
                n = 2 ** k
                base = n - 1
                get_level(k)
                hgat = lvl[k + 1]["hgat"]
                for j in range((n + NCHUNK - 1) // NCHUNK):
                    j0 = j * NCHUNK
                    w = min(NCHUNK, n - j0)
                    wp = max(w, 2)

                    slab = spool.tile([P, KX, 2 * wp], F8E3, tag="slab",
                                      name=f"sl{k}_{j}")
                    pw = hgat[0][1]
                    pos, off, need = 2 * j0, 0, 2 * w
                    while need > 0:
                        pj, pc = divmod(pos, pw)
                        take = min(need, pw - pc)
                        nc.gpsimd.dma_start(
                            slab[:, :, off:off + take],
                            hgat[pj][0][:, pc:pc + take].rearrange(
                                "(c p) w -> p c w", p=P
                            ),
                        )
                        pos += take; off += take; need -= take
                    if wp != w:
                        nc.gpsimd.dma_start(
                            slab[:, :, 2 * w:4 * w],
                            hgat[0][0][:, 0:2 * w].rearrange(
                                "(c p) w -> p c w", p=P
                            ),
                        )

                    small = k <= 6   # top tree: x from resident exS in PSUM
                    ps = [None] * 4
                    for q in range(4):
                        ps[q] = psum.tile([P, wp], F32, tag="ps", bufs=8,
                                          name=f"ps{k}_{j}_{q}")
                    if small:
                        # x matmuls first: independent of the AllGather, they
                        # keep the PE busy while the gather drains
                        for q in range(4):
                            for a in range(KX):
                                nc.tensor.matmul(
                                    ps[q][:],
                                    wih[:, a, q * P:(q + 1) * P],
                                    exS[:, a, base: base + wp],
                                    start=(a == 0), stop=False,
                                )
                    for q in range(4):
                        for c in range(KH):
                            nc.tensor.matmul(
                                ps[q][:],
                                whh[:, c, q * P:(q + 1) * P],
                                slab[:, c % KX, (c // KX)::2],
                                start=(not small and c == 0),
                                stop=(c == KH - 1),
                            )
                    if small:
                        emit_tail(k, j0, w, wp, ps, 1.0 / G_SCALE)
                    else:
                        # fold the precomputed x part in on the DVE
                        cmb = [None] * 4
                        for q in range(4):
                            ct = ewpool.tile([P, wp], F32, tag=f"cb{q}", bufs=1)
                            nc.vector.tensor_add(
                                ct[:], ps[q][:], xw[:, q, base + j0: base + j0 + wp]
                            )
                            cmb[q] = ct
                        emit_tail(k, j0, w, wp, cmb, 1.0 / G_SCALE)

    nc.compile()
    return nc


def _prep_inputs(emb, W_ih, W_hh, b_ih, b_hh):
    """Host-side sharding: kept-gate rows, per-core slices, transposes."""
    import ml_dtypes

    emb = np.asarray(emb, dtype=np.float32)
    W_ih = np.asarray(W_ih, dtype=np.float32)
    W_hh = np.asarray(W_hh, dtype=np.float32)
    b = np.asarray(b_ih, dtype=np.float32) + np.asarray(b_hh, dtype=np.float32)

    embT = np.ascontiguousarray(emb.T).astype(ml_dtypes.bfloat16)  # (I, 4095)
    # top-tree emb slice, pre-scaled x512 to match the scaled hh PSUM
    embS = np.ascontiguousarray(
        (emb[0:128, :] * G_SCALE).T
    ).astype(ml_dtypes.bfloat16)  # (I, 128)
    in_maps = []
    for m in range(NCORES):
        rows = np.concatenate(
            [np.arange(q * 2 * H + m * P, q * 2 * H + m * P + P) for q in range(4)]
        )
        wihT = np.ascontiguousarray(W_ih[rows, :].T).astype(ml_dtypes.bfloat16)
        whhT = np.ascontiguousarray(
            W_hh[rows, :].T * W_HH_SCALE
        ).astype(ml_dtypes.float8_e3m4)                     # (2H, 512)
        bias = np.ascontiguousarray(b[rows].reshape(4, P).T)  # (128, 4)
        brow = (b[rows] * G_SCALE).reshape(1, GS).astype(ml_dtypes.bfloat16)
        in_maps.append({"embT": embT, "embS": embS, "wihT": wihT,
                        "whhT": whhT, "bias": bias, "brow": brow,
                        "ones": np.ones((1, P), ml_dtypes.bfloat16),
                        "eye": np.eye(P, dtype=np.float32)})
    return in_maps


def _install_profile_hook():
    """The agent image's antenv lacks axon_hooks; synthesize it so
    run_bass_kernel_spmd(trace=True) can capture NTFF profiles."""
    import types

    if "antenv.axon_hooks" in sys.modules:
        return
    try:
        from trn_agent_boot.trn_boot import _ntff_profile_via_ctypes
    except ImportError:
        return
    hook = _ntff_profile_via_ctypes("/opt/axon/libaxon_pjrt.so")
    mod = types.ModuleType("antenv.axon_hooks")
    mod._hook = hook
    mod.set_axon_ntff_profile_hook = lambda h: setattr(mod, "_hook", h)
    mod.get_axon_ntff_profile_hook = lambda: mod._hook
    sys.modules["antenv.axon_hooks"] = mod
    import antenv

    antenv.axon_hooks = mod


def _run(in_maps, trace=False):
    if trace:
        _install_profile_hook()
    if "nc" not in _CACHE:
        _CACHE["nc"] = _build()
    nc = _CACHE["nc"]
    res = bass_utils.run_bass_kernel_spmd(
        nc, in_maps, core_ids=list(range(NCORES)), trace=trace
    )
    return res


def _assemble(results):
    out = np.zeros((1, 2 * H), dtype=np.float32)
    for m in range(NCORES):
        o = results[m]["out"].reshape(2 * P)
        out[0, m * P:(m + 1) * P] = o[0:P]
        out[0, H + m * P: H + (m + 1) * P] = o[P:2 * P]
    return out


def kernel(emb, W_ih, W_hh, b_ih, b_hh):
    in_maps = _prep_inputs(emb, W_ih, W_hh, b_ih, b_hh)
    res = _run(in_maps, trace=False)
    return _assemble(res.results)


# revision 17
# speedup vs baseline: 1.0653x; 1.0653x over previous
"""BinaryTreeLSTM on 8 TRN2 NeuronCores.

Strategy: tensor-parallel over the 8H gate dimension (sharding hint).
Key algebraic facts exploited:
  - The reference keeps only the first H dims of h_new/c_new per level, so
    only gate rows {q*2H + [0:H]} of the 8H weight rows ever matter
    ("kept gates": 4H instead of 8H -> 2x less matmul work).
  - c_cat[:, :H] is the LEFT child's c only, elementwise per hidden dim ->
    c never needs to be exchanged between cores; only h is all-gathered.
  - At the leaf level h = c = 0 -> the W_hh matmul and the f-gate*c term
    are skipped entirely.
Each core m owns hidden dims [128m, 128m+128) of each of the i,f,g,o gates
(a 512-wide gate slice). Per level it computes gates.T (feature-major:
gate dims on PSUM partitions, nodes on the free axis), applies the LSTM
cell elementwise, and all-gathers its h.T slice (128, n) into the full
h.T (1024, n) for the next level.

v2 vs baseline (433749 ns):
  - h stored/gathered as float8e3 (e3m4) scaled x8 -> AllGather bytes
    halved (the big leaf gathers were ~86us of serial cc time).
  - W_hh stored as e3m4 scaled x64 -> hh matmuls are fp8xfp8, half the
    LDWEIGHTS time, half the SBUF.  Gates come out x512; the scalar-engine
    activation computes func(in*1/512 + bias) for free.
  - x-path stays bf16 (fp8 there alone costs 2.7e-2 rel err - fails).
    Numpy end-to-end sim of this exact mix: rel err 7.9e-3 (budget 2e-2).
  - A dummy 1-column AllGather issued first absorbs the one-time ~38us
    collective BARRIER behind the weight DMAs + leaf compute.
"""

import sys

for p in ("/opt/trn_rl_repo",):
    if p not in sys.path:
        sys.path.insert(0, p)

import numpy as np

import concourse.bass as bass
import concourse.bacc as bacc
import concourse.mybir as mybir
import concourse.tile as tile
from concourse import bass_utils

H = 1024
I = 1024
DEPTH = 12
NCORES = 8
P = 128            # partitions / per-core hidden slice
GS = 4 * P         # per-core gate slice (i,f,g,o each P wide) = 512
NCHUNK = 512       # node-column chunk (PSUM bank = 512 fp32)
F32 = mybir.dt.float32
BF16 = mybir.dt.bfloat16
F8E3 = mybir.dt.float8e3
AF = mybir.ActivationFunctionType

W_HH_SCALE = 64.0   # W_hh quantization scale (e3m4)
H_SCALE = 8.0       # h quantization scale (e3m4)
G_SCALE = W_HH_SCALE * H_SCALE   # gates arrive x512 -> activation scale 1/512

_CACHE = {}


def _build():
    nc = bacc.Bacc(
        "TRN2",
        target_bir_lowering=False,
        debug=False,
        enable_asserts=False,
        num_devices=NCORES,
    )

    NTOT = 2 ** DEPTH - 1  # 4095
    embT_d = nc.dram_tensor("embT", (I, NTOT), BF16, kind="ExternalInput")
    embS_d = nc.dram_tensor("embS", (I, 128), BF16, kind="ExternalInput")
    wihT_d = nc.dram_tensor("wihT", (I, GS), BF16, kind="ExternalInput")
    whhT_d = nc.dram_tensor("whhT", (2 * H, GS), F8E3, kind="ExternalInput")
    bias_d = nc.dram_tensor("bias", (P, 4), F32, kind="ExternalInput")
    brow_d = nc.dram_tensor("brow", (1, GS), BF16, kind="ExternalInput")
    ones_d = nc.dram_tensor("ones", (1, P), BF16, kind="ExternalInput")
    eye_d = nc.dram_tensor("eye", (P, P), F32, kind="ExternalInput")
    out_d = nc.dram_tensor("out", (2 * P, 1), F32, kind="ExternalOutput")

    KX = I // P        # 8 contraction chunks for the x part
    KH = 2 * H // P    # 16 contraction chunks for the hh part
    rg = [list(range(NCORES))]

    with tile.TileContext(nc) as tc:
        with (
            tc.tile_pool(name="wpool", bufs=1) as wpool,
            tc.tile_pool(name="xpool", bufs=2) as xpool,
            tc.tile_pool(name="spool", bufs=2) as spool,
            tc.tile_pool(name="state", bufs=2) as state,
            tc.tile_pool(name="ewpool", bufs=2) as ewpool,
            tc.tile_pool(name="psum", bufs=8, space=bass.MemorySpace.PSUM) as psum,
            tc.tile_pool(name="dram", bufs=2, space=bass.MemorySpace.DRAM) as dram,
        ):
            # ---- dummy collective: absorb the one-time cc BARRIER ----
            dummy_in = dram.tile([P, 2], F32, name="dummy_in")
            dummy_out = dram.tile([NCORES * P, 2], F32, name="dummy_out",
                                  addr_space="Shared")
            nc.sync.dma_start(dummy_in[:], bias_d[:, 0:2])
            nc.gpsimd.collective_compute(
                "AllGather",
                mybir.AluOpType.bypass,
                replica_groups=rg,
                ins=[dummy_in.opt()],
                outs=[dummy_out.opt()],
            )

            # resident weights, feature-major: [:, c, q*128:(q+1)*128] is the
            # stationary (K=128, M=128) tile for contraction chunk c, gate q
            wih = wpool.tile([P, KX, GS], BF16)
            whh = wpool.tile([P, KH, GS], F8E3)
            bias = wpool.tile([P, 4], F32)
            brow = wpool.tile([1, GS], BF16)
            ones = wpool.tile([1, P], BF16)
            eye = wpool.tile([P, P], F32)
            for a in range(KX):
                nc.sync.dma_start(wih[:, a, :], wihT_d[a * P:(a + 1) * P, :])
            for c in range(KH):
                nc.scalar.dma_start(whh[:, c, :], whhT_d[c * P:(c + 1) * P, :])
            nc.sync.dma_start(bias[:], bias_d[:])
            nc.gpsimd.dma_start(brow[:], brow_d[:])
            nc.gpsimd.dma_start(ones[:], ones_d[:])
            nc.gpsimd.dma_start(eye[:], eye_d[:])

            # ---- phase structure ------------------------------------
            # 1. leaf level (k=11): x-only gates, elementwise, chunked
            #    AllGathers start flowing immediately.
            # 2. XW precompute: x@W_ih.T for ALL interior nodes (heap rows
            #    0..2046) into SBUF - dense PE work that runs while the
            #    leaf AllGathers drain, and removes the x part from the
            #    recurrent critical path entirely.  Stored x512 to match
            #    the scaled hh PSUM.
            # 3. levels 10..0: hh-only PSUM groups + xw combine + cell.
            xw = wpool.tile([P, 4, 2 * NCHUNK * 2], F32)  # (128, 4, 2048)
            # emb.T x512 for the tiny top-tree levels (heap rows 0..126)
            exS = wpool.tile([P, KX, 128], BF16)

            lvl = {}

            def get_level(k):
                if k not in lvl:
                    n = 2 ** k
                    h_new = state.tile(
                        [P, max(n, 2)], F8E3, tag="hst", bufs=2, name=f"h{k}"
                    )
                    c_new = state.tile(
                        [P, max(n, 2)], F32, tag="cst", bufs=3, name=f"c{k}"
                    )
                    lvl[k] = {"h": h_new, "c": c_new, "hgat": []}
                return lvl[k]

            def emit_tail(k, j0, w, wp, ps, gscale):
                """LSTM cell on finished gate tiles + chunked AllGather.

                ps tiles hold gates * (1/gscale is applied here); bias is in
                true units (activation computes func(in*scale + bias)).
                """
                L = lvl[k]
                h_new, c_new = L["h"], L["c"]
                sig_i = ewpool.tile([P, wp], F32, tag="si")
                tan_g = ewpool.tile([P, wp], F32, tag="tg")
                sig_o = ewpool.tile([P, wp], F32, tag="so")
                nc.scalar.activation(sig_i[:], ps[0][:], AF.Sigmoid,
                                     bias=bias[:, 0:1], scale=gscale)
                nc.scalar.activation(tan_g[:], ps[2][:], AF.Tanh,
                                     bias=bias[:, 2:3], scale=gscale)
                if k < DEPTH - 1:
                    sig_f = ewpool.tile([P, wp], F32, tag="sf")
                    nc.scalar.activation(sig_f[:], ps[1][:], AF.Sigmoid,
                                         bias=bias[:, 1:2], scale=gscale)
                nc.scalar.activation(sig_o[:], ps[3][:], AF.Sigmoid,
                                     bias=bias[:, 3:4], scale=gscale)

                t2 = ewpool.tile([P, wp], F32, tag="t2")
                nc.vector.tensor_mul(t2[:], sig_i[:], tan_g[:])
                if k < DEPTH - 1:
                    c_prev = lvl[k + 1]["c"]
                    if wp == w:
                        c_left = c_prev[:, 2 * j0: 2 * j0 + 2 * w: 2]
                    else:
                        c_left = c_prev[:, 0:2]
                    t1 = ewpool.tile([P, wp], F32, tag="t1")
                    nc.vector.tensor_mul(t1[:], sig_f[:], c_left)
                    nc.vector.tensor_add(c_new[:, j0:j0 + wp], t1[:], t2[:])
                else:
                    nc.vector.tensor_copy(c_new[:, j0:j0 + wp], t2[:])

                tan_c = ewpool.tile([P, wp], F32, tag="tc")
                nc.scalar.activation(tan_c[:], c_new[:, j0:j0 + wp], AF.Tanh)
                if k > 0:
                    t_h = ewpool.tile([P, wp], F32, tag="th")
                    nc.vector.tensor_mul(t_h[:], sig_o[:], tan_c[:])
                    # quantize h*8 -> e3m4 for the gather / hh matmul
                    nc.scalar.activation(h_new[:, j0:j0 + wp], t_h[:],
                                         AF.Copy, scale=H_SCALE)
                    for p0 in range(j0, j0 + w, 512):
                        pw = min(512, j0 + w - p0)
                        ag_in = dram.tile([P, pw], F8E3, tag="agin", bufs=6,
                                          name=f"agin{k}_{p0}")
                        ag_out = dram.tile([NCORES * P, pw], F8E3, tag="agout",
                                           bufs=10, name=f"agout{k}_{p0}",
                                           addr_space="Shared")
                        nc.sync.dma_start(ag_in[:], h_new[:, p0:p0 + pw])
                        nc.gpsimd.collective_compute(
                            "AllGather",
                            mybir.AluOpType.bypass,
                            replica_groups=rg,
                            ins=[ag_in.opt()],
                            outs=[ag_out.opt()],
                        )
                        L["hgat"].append((ag_out, pw))
                else:
                    h_root = ewpool.tile([P, 2], F32, tag="hroot")
                    nc.vector.tensor_mul(h_root[:], sig_o[:], tan_c[:])
                    nc.sync.dma_start(out_d[0:P, :], h_root[:, 0:1])
                    nc.sync.dma_start(out_d[P:2 * P, :], c_new[:, 0:1])

            # ---- phase 1: leaf level --------------------------------
            K = DEPTH - 1
            nl = 2 ** K
            get_level(K)
            for j in range(nl // NCHUNK):
                j0 = j * NCHUNK
                ex = xpool.tile([P, KX, NCHUNK], BF16, tag="ex", name=f"exL{j}")
                for a in range(KX):
                    nc.sync.dma_start(
                        ex[:, a, :],
                        embT_d[a * P:(a + 1) * P,
                               nl - 1 + j0: nl - 1 + j0 + NCHUNK],
                    )
                ps = [None] * 4
                for q in (0, 2, 3):
                    ps[q] = psum.tile([P, NCHUNK], F32, tag="ps", bufs=6,
                                      name=f"psL{j}_{q}")
                for q in (0, 2, 3):
                    for a in range(KX):
                        nc.tensor.matmul(
                            ps[q][:], wih[:, a, q * P:(q + 1) * P], ex[:, a, :],
                            start=(a == 0), stop=(a == KX - 1),
                        )
                emit_tail(K, j0, NCHUNK, NCHUNK, ps, 1.0)

            for a in range(KX):
                nc.gpsimd.dma_start(exS[:, a, :], embS_d[a * P:(a + 1) * P, :])

            # ---- phase 2: XW precompute for heap rows 0..2047 -------
            for j in range(4):
                j0 = j * NCHUNK
                ex = xpool.tile([P, KX, NCHUNK], BF16, tag="ex", name=f"exP{j}")
                nc.sync.dma_start(
                    ex[:],
                    embT_d[:, j0: j0 + NCHUNK].rearrange("(a p) w -> p a w", p=P),
                )
                for q in range(4):
                    pt = psum.tile([P, NCHUNK], F32, tag="ps", bufs=6,
                                   name=f"psP{j}_{q}")
                    for a in range(KX):
                        nc.tensor.matmul(
                            pt[:], wih[:, a, q * P:(q + 1) * P], ex[:, a, :],
                            start=(a == 0), stop=(a == KX - 1),
                        )
                    # store x512 so it adds directly onto the scaled hh PSUM
                    nc.scalar.activation(xw[:, q, j0:j0 + NCHUNK], pt[:],
                                         AF.Copy, scale=G_SCALE)

            # ---- phase 3: recurrent sweep, hh only ------------------
            for k in range(DEPTH - 2, -1, -1):
                n = 2 ** k
                base = n - 1
                get_level(k)
                hgat = lvl[k + 1]["hgat"]
                for j in range((n + NCHUNK - 1) // NCHUNK):
                    j0 = j * NCHUNK
                    w = min(NCHUNK, n - j0)
                    wp = max(w, 2)

                    slab = spool.tile([P, KX, 2 * wp], F8E3, tag="slab",
                                      name=f"sl{k}_{j}")
                    pw = hgat[0][1]
                    pos, off, need = 2 * j0, 0, 2 * w
                    while need > 0:
                        pj, pc = divmod(pos, pw)
                        take = min(need, pw - pc)
                        nc.gpsimd.dma_start(
                            slab[:, :, off:off + take],
                            hgat[pj][0][:, pc:pc + take].rearrange(
                                "(c p) w -> p c w", p=P
                            ),
                        )
                        pos += take; off += take; need -= take
                    if wp != w:
                        nc.gpsimd.dma_start(
                            slab[:, :, 2 * w:4 * w],
                            hgat[0][0][:, 0:2 * w].rearrange(
                                "(c p) w -> p c w", p=P
                            ),
                        )

                    small = k <= 6   # top tree: x from resident exS in PSUM
                    ps = [None] * 4
                    for q in range(4):
                        ps[q] = psum.tile([P, wp], F32, tag="ps", bufs=6,
                                          name=f"ps{k}_{j}_{q}")
                    if small:
                        # x matmuls first: independent of the AllGather, they
                        # keep the PE busy while the gather drains
                        for q in range(4):
                            for a in range(KX):
                                nc.tensor.matmul(
                                    ps[q][:],
                                    wih[:, a, q * P:(q + 1) * P],
                                    exS[:, a, base: base + wp],
                                    start=(a == 0), stop=False,
                                )
                        # keep the PE ramped through the gather wait: junk
                        # matmuls on resident tiles into a scratch bank (the
                        # PE drops to ~1.2GHz pstate after ~us of idling and
                        # then runs the real hh matmuls at half speed)
                        warm = psum.tile([P, GS], F32, tag="warm", bufs=1,
                                         name=f"warm{k}_{j}")
                        for a in range(6):
                            nc.tensor.matmul(
                                warm[:], exS[:, a, 0:P], wih[:, a, :],
                                start=True, stop=True,
                            )
                    for q in range(4):
                        for c in range(KH):
                            nc.tensor.matmul(
                                ps[q][:],
                                whh[:, c, q * P:(q + 1) * P],
                                slab[:, c % KX, (c // KX)::2],
                                start=(not small and c == 0),
                                stop=(c == KH - 1),
                            )
                    if small:
                        emit_tail(k, j0, w, wp, ps, 1.0 / G_SCALE)
                    else:
                        # fold the precomputed x part in on the DVE
                        cmb = [None] * 4
                        for q in range(4):
                            ct = ewpool.tile([P, wp], F32, tag=f"cb{q}", bufs=1)
                            nc.vector.tensor_add(
                                ct[:], ps[q][:], xw[:, q, base + j0: base + j0 + wp]
                            )
                            cmb[q] = ct
                        emit_tail(k, j0, w, wp, cmb, 1.0 / G_SCALE)

    nc.compile()
    return nc


def _prep_inputs(emb, W_ih, W_hh, b_ih, b_hh):
    """Host-side sharding: kept-gate rows, per-core slices, transposes."""
    import ml_dtypes

    emb = np.asarray(emb, dtype=np.float32)
    W_ih = np.asarray(W_ih, dtype=np.float32)
    W_hh = np.asarray(W_hh, dtype=np.float32)
    b = np.asarray(b_ih, dtype=np.float32) + np.asarray(b_hh, dtype=np.float32)

    embT = np.ascontiguousarray(emb.T).astype(ml_dtypes.bfloat16)  # (I, 4095)
    # top-tree emb slice, pre-scaled x512 to match the scaled hh PSUM
    embS = np.ascontiguousarray(
        (emb[0:128, :] * G_SCALE).T
    ).astype(ml_dtypes.bfloat16)  # (I, 128)
    in_maps = []
    for m in range(NCORES):
        rows = np.concatenate(
            [np.arange(q * 2 * H + m * P, q * 2 * H + m * P + P) for q in range(4)]
        )
        wihT = np.ascontiguousarray(W_ih[rows, :].T).astype(ml_dtypes.bfloat16)
        whhT = np.ascontiguousarray(
            W_hh[rows, :].T * W_HH_SCALE
        ).astype(ml_dtypes.float8_e3m4)                     # (2H, 512)
        bias = np.ascontiguousarray(b[rows].reshape(4, P).T)  # (128, 4)
        brow = (b[rows] * G_SCALE).reshape(1, GS).astype(ml_dtypes.bfloat16)
        in_maps.append({"embT": embT, "embS": embS, "wihT": wihT,
                        "whhT": whhT, "bias": bias, "brow": brow,
                        "ones": np.ones((1, P), ml_dtypes.bfloat16),
                        "eye": np.eye(P, dtype=np.float32)})
    return in_maps


def _install_profile_hook():
    """The agent image's antenv lacks axon_hooks; synthesize it so
    run_bass_kernel_spmd(trace=True) can capture NTFF profiles."""
    import types

    if "antenv.axon_hooks" in sys.modules:
        return
    try:
        from trn_agent_boot.trn_boot import _ntff_profile_via_ctypes
    except ImportError:
        return
    hook = _ntff_profile_via_ctypes("/opt/axon/libaxon_pjrt.so")
    mod = types.ModuleType("antenv.axon_hooks")
    mod._hook = hook
    mod.set_axon_ntff_profile_hook = lambda h: setattr(mod, "_hook", h)
    mod.get_axon_ntff_profile_hook = lambda: mod._hook
    sys.modules["antenv.axon_hooks"] = mod
    import antenv

    antenv.axon_hooks = mod


def _run(in_maps, trace=False):
    if trace:
        _install_profile_hook()
    if "nc" not in _CACHE:
        _CACHE["nc"] = _build()
    nc = _CACHE["nc"]
    res = bass_utils.run_bass_kernel_spmd(
        nc, in_maps, core_ids=list(range(NCORES)), trace=trace
    )
    return res


def _assemble(results):
    out = np.zeros((1, 2 * H), dtype=np.float32)
    for m in range(NCORES):
        o = results[m]["out"].reshape(2 * P)
        out[0, m * P:(m + 1) * P] = o[0:P]
        out[0, H + m * P: H + (m + 1) * P] = o[P:2 * P]
    return out


def kernel(emb, W_ih, W_hh, b_ih, b_hh):
    in_maps = _prep_inputs(emb, W_ih, W_hh, b_ih, b_hh)
    res = _run(in_maps, trace=False)
    return _assemble(res.results)
